# revision 1
# baseline (speedup 1.0000x reference)
"""Trainium2 Bass kernel: batched tiny-window attention (B=6272, N=8, C=768, H=12).

Data-parallel over 8 NeuronCores (784 batches / 6272 tokens per core).
Per-core fused pipeline, fp16 compute, fp32 accumulate:
  x -> (PE transpose) xT[c,tok] -> qkv matmul -> qT/kT [oc,tok] + v [tok,oc]
    -> per-128-token-group block-diag attention, 4 heads per PSUM bank:
       PSUM pre-seeded with the additive log-bias mask via an identity
       matmul, S accumulated on top, ACT exp with fused masked row-sum,
       GpSimd row normalization, PE transpose of A, out_h = v_h.T @ AT
    -> attnT[c,tok] -> proj matmul -> out [tok, C] -> DMA.
Scale (hd^-0.5) and qkv bias are folded into host-side precomputed weights.
"""

import os
import sys
from contextlib import ExitStack

import numpy as np

sys.path.insert(0, "/opt/trn_rl_repo")

import concourse.bass as bass  # noqa: E402
import concourse.bacc as bacc  # noqa: E402
import concourse.tile as tile  # noqa: E402
from concourse import mybir  # noqa: E402
from concourse.bass_utils import run_bass_kernel_spmd  # noqa: E402
from concourse.masks import make_identity  # noqa: E402
from concourse.tile import add_dep_helper  # noqa: E402

NCORES = 8
B, N, C = 6272, 8, 768
H, HD = 12, 64
OC = 3 * C
B_LOC = B // NCORES          # 784 batches per core
TOK = B_LOC * N              # 6272 tokens per core
CCH = C // 128               # 6 channel chunks
GRP = 128                    # tokens per attention group (16 batches)
MACRO = 512                  # tokens per macro tile
NQ = H // 4                  # head quads

F16 = mybir.dt.float16
F32 = mybir.dt.float32

LAST_RESULT = {}             # test harness introspection (exec_time_ns etc.)


def _build_nc(use_bias: bool):
    nc = bacc.Bacc()
    x_ext = nc.declare_dram_parameter("x", [TOK, C], F32, isOutput=False)
    wqkv_ext = nc.declare_dram_parameter("wqkvT", [C, OC], F16, isOutput=False)
    wproj_ext = nc.declare_dram_parameter("wprojT", [C, C], F16, isOutput=False)
    lm_ext = nc.declare_dram_parameter("lmask", [H, GRP, GRP], F16, isOutput=False)
    if use_bias:
        qkb_ext = nc.declare_dram_parameter("qkb", [2 * C], F32, isOutput=False)
        vb_ext = nc.declare_dram_parameter("vb", [C], F32, isOutput=False)
    out_ext = nc.declare_dram_parameter("out", [TOK, C], F32, isOutput=True)

    macros = []
    t0 = 0
    while t0 < TOK:
        tw = min(MACRO, TOK - t0)
        macros.append((t0, tw))
        t0 += tw

    with tile.TileContext(nc) as tc, ExitStack() as ctx:
        wpool = ctx.enter_context(tc.tile_pool(name="weights", bufs=1))
        xf32p = ctx.enter_context(tc.tile_pool(name="xf32", bufs=8))
        xTp = ctx.enter_context(tc.tile_pool(name="xT", bufs=12))
        qkTp = ctx.enter_context(tc.tile_pool(name="qkT", bufs=24))
        vp = ctx.enter_context(tc.tile_pool(name="v", bufs=8))
        attp = ctx.enter_context(tc.tile_pool(name="attnT", bufs=12))
        smallp = ctx.enter_context(tc.tile_pool(name="small", bufs=16))
        statp = ctx.enter_context(tc.tile_pool(name="stat", bufs=16))
        outp = ctx.enter_context(tc.tile_pool(name="outsb", bufs=4))
        # PSUM budget (8 banks): qk/xT 2, v/proj 2, attention 4
        ps_bqk = ctx.enter_context(tc.tile_pool(name="ps_bqk", bufs=2, space="PSUM"))
        ps_bvp = ctx.enter_context(tc.tile_pool(name="ps_bvp", bufs=1, space="PSUM"))
        ps_att = ctx.enter_context(tc.tile_pool(name="ps_att", bufs=3, space="PSUM"))
        ps_att2 = ctx.enter_context(tc.tile_pool(name="ps_att2", bufs=2, space="PSUM"))

        # --- persistent weights / masks / identities ---
        id_f32 = wpool.tile([128, 128], F32)
        make_identity(nc, id_f32)
        id_f16 = wpool.tile([128, 128], F16)
        make_identity(nc, id_f16)

        wqkv = []
        for c in range(CCH):
            wt = wpool.tile([128, OC], F16, tag=f"wqkv{c}", name="wt")
            nc.sync.dma_start(out=wt, in_=wqkv_ext.ap()[c * 128:(c + 1) * 128, :])
            wqkv.append(wt)
        wproj = []
        for c in range(CCH):
            wt = wpool.tile([128, C], F16, tag=f"wproj{c}", name="wt")
            nc.sync.dma_start(out=wt, in_=wproj_ext.ap()[c * 128:(c + 1) * 128, :])
            wproj.append(wt)
        lmask = []
        for h in range(H):
            lt = wpool.tile([128, 128], F16, tag=f"lmask{h}", name="lt")
            nc.sync.dma_start(out=lt, in_=lm_ext.ap()[h])
            lmask.append(lt)

        qkb_t = vb_t = None
        if use_bias:
            qkb_t = wpool.tile([128, 2 * CCH], F32)
            nc.sync.dma_start(
                out=qkb_t, in_=qkb_ext.ap().rearrange("(a p) -> p a", p=128))
            vb_t = wpool.tile([128, C], F32)
            nc.sync.dma_start(out=vb_t, in_=vb_ext.ap().to_broadcast((128, C)))

        def emit_ab(t0, tw):
            """Phases A+B: x load/transpose, qkv matmuls.  Returns state."""
            nsub = tw // GRP
            xT = [xTp.tile([128, MACRO], F16, tag="xt", name="xt")
                  for _ in range(CCH)]
            xin = [xf32p.tile([128, C], F32, tag="xin", name="xin")
                   for _ in range(nsub)]
            for s in range(nsub):
                nc.sync.dma_start(
                    out=xin[s], in_=x_ext.ap()[t0 + s * GRP: t0 + (s + 1) * GRP, :])
            for c in range(CCH):
                pst = ps_bqk.tile([128, 512], F32, tag="bqk", name="pst")
                for s in range(nsub):
                    nc.tensor.transpose(
                        out=pst[:, s * GRP:(s + 1) * GRP],
                        in_=xin[s][:, c * 128:(c + 1) * 128], identity=id_f32)
                nc.vector.tensor_copy(out=xT[c][:, :tw], in_=pst[:, :tw])

            qkT = [qkTp.tile([128, MACRO], F16, tag="qkt", name="qkt")
                   for _ in range(2 * CCH)]
            for j in range(2 * CCH):
                psq = ps_bqk.tile([128, 512], F32, tag="bqk", name="psq")
                for c in range(CCH):
                    nc.tensor.matmul(
                        psq[:, :tw],
                        lhsT=wqkv[c][:, j * 128:(j + 1) * 128],
                        rhs=xT[c][:, :tw],
                        start=(c == 0), stop=(c == CCH - 1))
                if use_bias:
                    nc.vector.tensor_scalar(
                        out=qkT[j][:, :tw], in0=psq[:, :tw],
                        scalar1=qkb_t[:, j:j + 1], scalar2=None,
                        op0=mybir.AluOpType.add)
                else:
                    nc.vector.tensor_copy(out=qkT[j][:, :tw], in_=psq[:, :tw])

            vt = [vp.tile([128, C], F16, tag="vt", name="vt") for _ in range(nsub)]
            for s in range(nsub):
                for g in range(2):
                    psv = ps_bvp.tile([128, 512], F32, tag="bvp", name="psv")
                    for c in range(CCH):
                        nc.tensor.matmul(
                            psv[:, 0:384],
                            lhsT=xT[c][:, s * GRP:(s + 1) * GRP],
                            rhs=wqkv[c][:, 2 * C + 384 * g:2 * C + 384 * (g + 1)],
                            start=(c == 0), stop=(c == CCH - 1))
                    if use_bias:
                        nc.vector.tensor_tensor(
                            out=vt[s][:, 384 * g:384 * (g + 1)],
                            in0=psv[:, 0:384],
                            in1=vb_t[:, 384 * g:384 * (g + 1)],
                            op=mybir.AluOpType.add)
                    else:
                        nc.vector.tensor_copy(
                            out=vt[s][:, 384 * g:384 * (g + 1)], in_=psv[:, 0:384])
            return (t0, tw, nsub, qkT, vt)

        def emit_cd(st):
            """Phases C+D: attention + proj for a macro emitted earlier."""
            t0, tw, nsub, qkT, vt = st
            attnT = [attp.tile([128, MACRO], F16, tag="att", name="att")
                     for _ in range(CCH)]
            for s in range(nsub):
                gsl = slice(s * GRP, (s + 1) * GRP)
                # Wave 1: all score matmuls + exp + normalization
                a_n, rc2s = [], []
                for p in range(H // 2):     # head pair (2p, 2p+1)
                    rs2 = statp.tile([128, 2], F32, tag="rs", name="rs2")
                    for half in range(2):
                        h = 2 * p + half
                        psl = slice(64 * half, 64 * half + 64)
                        sq = ps_att.tile([128, 128], F32, tag="attps", name="sq")
                        nc.tensor.matmul(sq, lhsT=id_f16, rhs=lmask[h],
                                         start=True, stop=False)
                        nc.tensor.matmul(sq, lhsT=qkT[p][psl, gsl],
                                         rhs=qkT[CCH + p][psl, gsl],
                                         start=False, stop=True)
                        a_t = smallp.tile([128, 128], F16, tag="a", name="a_t")
                        nc.scalar.activation(
                            out=a_t, in_=sq,
                            func=mybir.ActivationFunctionType.Exp,
                            accum_out=rs2[:, half:half + 1])
                        a_n.append(a_t)
                    rc2 = statp.tile([128, 2], F32, tag="rc", name="rc2")
                    nc.vector.reciprocal(out=rc2, in_=rs2)
                    rc2s.append(rc2)
                # Wave 2: normalize + transpose + MM2 per pair
                for p in range(H // 2):
                    at2 = ps_att2.tile([128, 256], F16, tag="at2", name="at2")
                    for half in range(2):
                        an = smallp.tile([128, 128], F16, tag="an", name="an")
                        nc.vector.tensor_scalar(
                            out=an, in0=a_n[2 * p + half],
                            scalar1=rc2s[p][:, half:half + 1],
                            scalar2=None, op0=mybir.AluOpType.mult)
                        nc.tensor.transpose(
                            out=at2[:, half * 128:(half + 1) * 128], in_=an,
                            identity=id_f16)
                    at2s = smallp.tile([128, 256], F16, tag="at2s", name="at2s")
                    nc.scalar.copy(out=at2s, in_=at2)
                    # MM2 pair-packed: 2 writes (partition halves) + 1 reader
                    op2 = ps_att2.tile([128, 128], F32, tag="at2", name="op2")
                    for half in range(2):
                        h = 2 * p + half
                        nc.tensor.matmul(
                            op2[64 * half:64 * (half + 1), :],
                            lhsT=vt[s][:, h * 64:(h + 1) * 64],
                            rhs=at2s[:, half * 128:(half + 1) * 128],
                            start=True, stop=True,
                            tile_position=(0, 64 * half))
                    nc.scalar.copy(out=attnT[p][:, gsl], in_=op2)

                # Phase D for this sub-tile
                osb = outp.tile([128, C], F32, tag="osb")
                for g in range(2):
                    psp = ps_bvp.tile([128, 512], F32, tag="bvp", name="psp")
                    for c in range(CCH):
                        nc.tensor.matmul(
                            psp[:, 0:384],
                            lhsT=attnT[c][:, s * GRP:(s + 1) * GRP],
                            rhs=wproj[c][:, 384 * g:384 * (g + 1)],
                            start=(c == 0), stop=(c == CCH - 1))
                    nc.vector.tensor_copy(
                        out=osb[:, 384 * g:384 * (g + 1)], in_=psp[:, 0:384])
                nc.sync.dma_start(
                    out=out_ext.ap()[t0 + s * GRP: t0 + (s + 1) * GRP, :], in_=osb)

        # Two-stage software pipeline: macro m's attention/proj is emitted
        # after macro m+1's qkv, so the PE always has independent work.
        pending = None
        for (t0, tw) in macros:
            st = emit_ab(t0, tw)
            if pending is not None:
                emit_cd(pending)
            pending = st
        emit_cd(pending)

    nc.compile()
    return nc


def make_host_inputs(qkv_w, qkv_b, proj_w, rel_bias_table):
    """Precompute device-side weight/mask layouts (fp16, scale folded)."""
    scale = HD ** -0.5
    wq = qkv_w.copy()
    wq[:C] *= scale
    bq = qkv_b.copy()
    bq[:C] *= scale
    wqkvT = np.ascontiguousarray(wq.T).astype(np.float16)          # [C, 3C]
    wprojT = np.ascontiguousarray(proj_w.T).astype(np.float16)     # [C, C]

    # Additive log-mask per head: in-block rel bias, off-block -30000
    # (exp -> 0 in fp32/fp16).
    lm = np.full((H, GRP, GRP), -30000.0, np.float32)
    for b in range(GRP // N):
        for n in range(N):
            for m in range(N):
                lm[:, b * N + n, b * N + m] = rel_bias_table[m - n + N - 1, :]
    lmask = lm.astype(np.float16)
    return wqkvT, wprojT, lmask, bq


_NC_CACHE = None


def kernel(x, qkv_w, qkv_b, proj_w, proj_b, rel_bias_table):
    global _NC_CACHE
    x = np.asarray(x, np.float32)
    qkv_w = np.asarray(qkv_w, np.float32)
    qkv_b = np.asarray(qkv_b, np.float32)
    proj_w = np.asarray(proj_w, np.float32)
    proj_b = np.asarray(proj_b, np.float32)
    tbl = np.asarray(rel_bias_table, np.float32)

    wqkvT, wprojT, lmask, bq = make_host_inputs(qkv_w, qkv_b, proj_w, tbl)

    use_bias = bool(np.any(qkv_b != 0))
    xs = x.reshape(NCORES, TOK, C)
    in_maps = []
    for i in range(NCORES):
        m = {"x": np.ascontiguousarray(xs[i]), "wqkvT": wqkvT,
             "wprojT": wprojT, "lmask": lmask}
        if use_bias:
            m["qkb"] = np.ascontiguousarray(bq[:2 * C])
            m["vb"] = np.ascontiguousarray(qkv_b[2 * C:])
        in_maps.append(m)

    if _NC_CACHE is None or _NC_CACHE[0] != use_bias:
        _NC_CACHE = (use_bias, _build_nc(use_bias))
    nc = _NC_CACHE[1]

    trace = bool(int(os.environ.get("KERNEL_TRACE", "0")))
    res = run_bass_kernel_spmd(nc, in_maps, core_ids=list(range(NCORES)),
                               trace=trace)
    LAST_RESULT["exec_time_ns"] = getattr(res, "exec_time_ns", None)
    LAST_RESULT["res"] = res
    out = np.concatenate([np.asarray(r["out"]) for r in res.results], axis=0)
    out = out.reshape(B, N, C).astype(np.float32)
    if np.any(proj_b != 0):
        out = out + proj_b[None, None, :]
    return out

